# revision 17
# baseline (speedup 1.0000x reference)
"""GQA attention kernel for 8 Trainium2 NeuronCores.

Sharding: core = (batch b, kv_group g), b in {0,1}, g in {0..3}.
Each core computes the 4 heads of one KV group for one batch and the
partial output projection for those heads; the host sums the 4 group
partials per batch.  Zero duplicated compute across cores.

All matmul operands are bf16 (fp32 PSUM accumulation); verified to give
~6e-3 max rel err vs the fp32 reference (tolerance 2e-2).

Performance structure (the tensor engine is the bottleneck; the whole
kernel keeps its queue dense and stall-free — ~87% MFU):
  - phase 1 (QKV projections): 6 concurrent PSUM accumulation groups;
    sc0 runs all Q matmuls first since wq/x arrive on the DMA queues
    before wk/wv.  x-tiles stream on the SP DMA queue, weights on the
    ACT-engine DMA queue (DMA instructions block their issuing engine
    for the whole transfer, so placement matters), V is transposed by
    the DMA xbar.  wk/wv are host-prearranged partition-major; x/wq
    keep strided patterns — an all-contiguous layout was measured to
    trigger a chip-wide ~20% DVFS clock drop.
  - phase 2 (attention): one flat software-pipelined stream across all
    (head-pair, kv-tile) steps: per step 2 scores matmuls into one
    2-bank PSUM tile, ONE exp (halves ACT per-op overhead; ACT is
    ~86% busy), and the sums/AV matmuls of the step TWO back, so exp
    latency hides behind ~8 matmuls and pair boundaries are seamless.
  - softmax: denominators via ones-stationary matmul (partition
    reduction on PE); PSUM banks freed by fast DVE/ACT copies in
    bank-reuse order; reciprocal + normalize run off-critical-path.
    The last pair's normalize is deferred into phase 3 (writing a
    separate tile — outT subtile deps would stall all of phase 3) and
    its sc<=1 region routes PSUM drains to ACT / outputs to sync so
    the deferred DVE chain never blocks PE.
  - phase 3 (output projection): transposed accumulation overlapping
    phase 2's drain tail; bf16 partial outputs on both DMA queues.
"""

import numpy as np

# problem shape (hardcoded per contract)
B, S, E = 2, 2048, 2048
H, G, D = 16, 4, 128
R = H // G          # heads per kv group = 4
ST = S // 128       # 16 t-tiles
ET = E // 128       # 16 e-tiles
SC = S // 512       # 4 s-chunks
QC = S // 512       # 4 q-chunks
NO = R + 2          # projection outputs per e-tile: 4x Q slices, K, V

_cache = {}


def _split_multi_waits(nc, maxw=1):
    """Walrus in this container accepts only one sync-wait per
    instruction; move extra waits onto preceding same-engine NoOps."""
    from concourse import mybir

    n_split = 0
    for fn in nc.m.functions:
        for bb in fn.blocks:
            out = []
            changed = False
            for inst in bb.instructions:
                si = inst.sync_info
                waits = list(si.on_wait or []) if si is not None else []
                if len(waits) > maxw:
                    changed = True
                    n_split += 1
                    head, tail = waits[:-maxw], waits[-maxw:]
                    for j in range(0, len(head), maxw):
                        nop = mybir.InstNoOp(
                            name=f"{inst.name}-wsplit{j}", ins=[], outs=[]
                        )
                        nop.engine = inst.engine
                        nop.sync_info = mybir.SyncInfo(
                            on_wait=head[j : j + maxw], on_update=[]
                        )
                        out.append(nop)
                    si.on_wait = tail
                out.append(inst)
            if changed:
                bb.instructions = out
    return n_split


def _build_program():
    import contextlib

    import concourse.bass as bass
    import concourse.tile as tile
    from concourse import mybir

    BF16 = mybir.dt.bfloat16
    F32 = mybir.dt.float32
    Exp = mybir.ActivationFunctionType.Exp
    Mult = mybir.AluOpType.mult

    nc = bass.Bass(target_bir_lowering=False)

    xT = nc.dram_tensor("xT", [E, S], BF16, kind="ExternalInput")
    wq = nc.dram_tensor("wq", [E, R * D], BF16, kind="ExternalInput")
    # wk/wv host-prearranged partition-major: their natural layout would
    # DMA as 256B rows at a fraction of peak
    wkh = nc.dram_tensor("wkh", [128, ET, D], BF16, kind="ExternalInput")
    wvh = nc.dram_tensor("wvh", [128, ET, D], BF16, kind="ExternalInput")
    wo = nc.dram_tensor("wo", [R * D, E], BF16, kind="ExternalInput")
    bqv = nc.dram_tensor("bqv", [R * D], F32, kind="ExternalInput")
    bkv = nc.dram_tensor("bkv", [D], F32, kind="ExternalInput")
    bvv = nc.dram_tensor("bvv", [D], F32, kind="ExternalInput")
    otd = nc.dram_tensor("ot", [E, S], BF16, kind="ExternalOutput")

    with tile.TileContext(nc) as tc:
        with contextlib.ExitStack() as ctx:
            consts = ctx.enter_context(tc.tile_pool(name="consts", bufs=1))
            big = ctx.enter_context(tc.tile_pool(name="big", bufs=1))

            bq_sb = consts.tile([128, R], F32)
            nc.gpsimd.dma_start(bq_sb, bqv.rearrange("(o p) -> p o", p=128))
            bk_sb = consts.tile([128, 1], F32)
            nc.gpsimd.dma_start(bk_sb, bkv.rearrange("(o p) -> p o", p=128))
            bv_sb = consts.tile([128, 1], F32)
            nc.gpsimd.dma_start(bv_sb, bvv.rearrange("(o p) -> p o", p=128))

            ones_f = consts.tile([128, 128], F32)
            nc.gpsimd.memset(ones_f, 1.0)
            ones = consts.tile([128, 128], BF16)
            nc.vector.tensor_copy(ones, ones_f)

            QT = big.tile([128, R, S], BF16)    # QT[d, h, q]
            KT = big.tile([128, S], BF16)       # KT[d, t]
            VT = big.tile([128, S], BF16)       # VT[d, t]
            V = big.tile([128, ST, D], BF16)    # V[t%128, tt, d]
            outT = big.tile([128, R, S], BF16)  # normalized attn out
            # last pair's normalized output lands here so phase-3's early
            # groups don't inherit a dependency on the deferred normalize
            outT_last = big.tile([128, 2, 512], BF16)
            wo_sb = big.tile([128, R, E], BF16)

            # ---- phase 1: QKV^T projections ----
            with tc.tile_pool(name="wts", bufs=1) as wpool, \
                 tc.tile_pool(name="xts", bufs=2) as xtpool, \
                 tc.tile_pool(name="ps1", bufs=8, space="PSUM") as ps1:
                wq_sb = wpool.tile([128, ET, R * D], BF16)
                wk_sb = wpool.tile([128, ET, D], BF16)
                wv_sb = wpool.tile([128, ET, D], BF16)
                # weights ride the ACT-engine DMA queue in multi-e-tile
                # chunks (per-DMA overhead ~0.6us dominates small transfers);
                # x-tiles ride the SP queue, also chunked
                xt0 = xtpool.tile([128, ET, 512], BF16, tag="xt")

                def _wchunk(dst, src_t, e0, e1):
                    nc.scalar.dma_start(
                        dst[:, e0:e1],
                        src_t[e0 * 128 : e1 * 128, :].rearrange(
                            "(o p) m -> p o m", p=128
                        ),
                    )

                _wchunk(wq_sb, wq, 0, 2)
                _wchunk(wq_sb, wq, 2, 6)
                nc.scalar.dma_start(wk_sb, wkh[:, :])
                _wchunk(wq_sb, wq, 6, 10)
                nc.scalar.dma_start(wv_sb, wvh[:, :])
                _wchunk(wq_sb, wq, 10, 14)
                _wchunk(wq_sb, wq, 14, 16)
                for k in range(4):
                    nc.sync.dma_start(
                        xt0[:, k * 4 : (k + 1) * 4],
                        xT[k * 512 : (k + 1) * 512, 0:512].rearrange(
                            "(o p) m -> p o m", p=128
                        ),
                    )

                for sc in range(SC):
                    if sc == 0:
                        xtile = xt0
                    else:
                        xtile = xtpool.tile([128, ET, 512], BF16, tag="xt")
                        for k in range(4):
                            nc.sync.dma_start(
                                xtile[:, k * 4 : (k + 1) * 4],
                                xT[k * 512 : (k + 1) * 512,
                                   sc * 512 : (sc + 1) * 512].rearrange(
                                    "(o p) m -> p o m", p=128
                                ),
                            )
                    cs = slice(sc * 512, (sc + 1) * 512)
                    pss = [ps1.tile([128, 512], F32, tag="p1",
                                    name=f"p1_{sc}_{i}")
                           for i in range(NO)]
                    # sc0 runs all Q matmuls first: wq/x arrive on the
                    # queues before wk/wv, so compute starts ~3us earlier
                    if sc == 0:
                        ot_passes = [(0, 1, 2, 3), (R, R + 1)]
                    else:
                        ot_passes = [(R, R + 1, 0, 1, 2, 3)]
                    for ots in ot_passes:
                        for e in range(ET):
                            for ot in ots:
                                if ot < R:
                                    lhsT = wq_sb[:, e,
                                                 ot * 128 : (ot + 1) * 128]
                                elif ot == R:
                                    lhsT = wk_sb[:, e]
                                else:
                                    lhsT = wv_sb[:, e]
                                nc.tensor.matmul(
                                    pss[ot], lhsT, xtile[:, e],
                                    start=(e == 0), stop=(e == ET - 1),
                                )
                    for ot in range(NO):
                        if ot < R:
                            dst, b = QT[:, ot, cs], bq_sb[:, ot : ot + 1]
                        elif ot == R:
                            dst, b = KT[:, cs], bk_sb[:, 0:1]
                        else:
                            dst, b = VT[:, cs], bv_sb[:, 0:1]
                        if ot % 2 == 0:
                            nc.scalar.add(dst, pss[ot], b)
                        else:
                            nc.vector.tensor_scalar_add(dst, pss[ot], b)
                    # V transpose for this chunk's 4 t-tiles via the DMA xbar
                    for tt in range(sc * 4, sc * 4 + 4):
                        nc.sync.dma_start_transpose(
                            V[:, tt], VT[:, tt * 128 : (tt + 1) * 128]
                        )

                # wo is needed only in phase 3; SP queue is idle by then
                nc.sync.dma_start(wo_sb, wo.rearrange("(o p) m -> p o m", p=128))

            # ---- phase 2: attention as one continuous stream ----
            # Steps s: scores+exp for stream-pair step s, consume (sums/AV)
            # matmuls for step s-2.  Uniform 6 matmuls per step, across pair
            # boundaries too, so the PE never outruns exp or PSUM rotation.
            ppool = ctx.enter_context(tc.tile_pool(name="probs", bufs=4))
            spool = ctx.enter_context(tc.tile_pool(name="ssb", bufs=4))
            avspool = ctx.enter_context(tc.tile_pool(name="avsb", bufs=4))
            rpool = ctx.enter_context(tc.tile_pool(name="rcs", bufs=4))
            with tc.tile_pool(name="ps_sc", bufs=2, space="PSUM") as scpool, \
                 tc.tile_pool(name="ps_sum", bufs=2, space="PSUM") as smpool, \
                 tc.tile_pool(name="ps_av", bufs=2, space="PSUM") as avpool:
                pairs = [(qc, hp) for qc in range(QC) for hp in range(R // 2)]
                NP = len(pairs)
                pts = {}
                acc = {}
                Cp = mybir.ActivationFunctionType.Copy
                for s in range(NP * ST + 2):
                    if s < NP * ST:
                        j, tt = divmod(s, ST)
                        qc, hp = pairs[j]
                        qs = slice(qc * 512, (qc + 1) * 512)
                        hA, hB = 2 * hp, 2 * hp + 1
                        ks = KT[:, tt * 128 : (tt + 1) * 128]
                        psc = scpool.tile([128, 2, 512], F32, tag="pss",
                                          name=f"psc_{s}")
                        nc.tensor.matmul(psc[:, 0], ks, QT[:, hA, qs],
                                         start=True, stop=True)
                        nc.tensor.matmul(psc[:, 1], ks, QT[:, hB, qs],
                                         start=True, stop=True)
                        pt = ppool.tile([128, 2, 512], BF16, tag="pt",
                                        name=f"pt_{s}")
                        nc.scalar.activation(pt, psc, Exp)
                        pts[s] = pt
                    c = s - 2
                    if c >= 0:
                        jc, ttc = divmod(c, ST)
                        if ttc == 0:
                            acc[jc] = (
                                smpool.tile([128, 512], F32, tag="sums",
                                            name=f"sumA_{jc}"),
                                smpool.tile([128, 512], F32, tag="sums",
                                            name=f"sumB_{jc}"),
                                avpool.tile([128, 512], F32, tag="av",
                                            name=f"avA_{jc}"),
                                avpool.tile([128, 512], F32, tag="av",
                                            name=f"avB_{jc}"),
                            )
                        sums_A, sums_B, av_A, av_B = acc[jc]
                        ptc = pts.pop(c)
                        st_, sp_ = (ttc == 0), (ttc == ST - 1)
                        nc.tensor.matmul(sums_A, ones, ptc[:, 0],
                                         start=st_, stop=sp_)
                        nc.tensor.matmul(av_A, V[:, ttc], ptc[:, 0],
                                         start=st_, stop=sp_)
                        nc.tensor.matmul(sums_B, ones, ptc[:, 1],
                                         start=st_, stop=sp_)
                        nc.tensor.matmul(av_B, V[:, ttc], ptc[:, 1],
                                         start=st_, stop=sp_)
                        if ttc == ST - 1:
                            # drain + normalize for pair jc, off critical path
                            qc, hp = pairs[jc]
                            qs = slice(qc * 512, (qc + 1) * 512)
                            hA, hB = 2 * hp, 2 * hp + 1
                            last = jc == NP - 1
                            ssA = spool.tile([128, 512], F32, tag="ssb",
                                             name=f"ssA_{jc}")
                            avsA = avspool.tile([128, 512], BF16, tag="avsb",
                                                name=f"avsA_{jc}")
                            ssB = spool.tile([128, 512], F32, tag="ssb",
                                             name=f"ssB_{jc}")
                            avsB = avspool.tile([128, 512], BF16, tag="avsb",
                                                name=f"avsB_{jc}")
                            if last:
                                # split drains ACT/DVE so phase-3 PSUM banks
                                # free fast; recip+normalize deferred into
                                # phase 3 (pool-close must not wait on them)
                                nc.scalar.activation(ssA, sums_A, Cp)
                                nc.scalar.activation(avsA, av_A, Cp)
                                nc.vector.tensor_copy(ssB, sums_B)
                                nc.vector.tensor_copy(avsB, av_B)
                                deferred = (hA, hB, qs, ssA, avsA, ssB, avsB)
                            else:
                                nc.vector.tensor_copy(ssA, sums_A)
                                nc.vector.tensor_copy(avsA, av_A)
                                nc.vector.tensor_copy(ssB, sums_B)
                                nc.vector.tensor_copy(avsB, av_B)
                                rcA = rpool.tile([128, 512], F32, tag="rc",
                                                 name=f"rcA_{jc}")
                                nc.vector.reciprocal(rcA, ssA)
                                nc.vector.tensor_tensor(outT[:, hA, qs], avsA,
                                                        rcA, Mult)
                                rcB = rpool.tile([128, 512], F32, tag="rc",
                                                 name=f"rcB_{jc}")
                                nc.vector.reciprocal(rcB, ssB)
                                nc.vector.tensor_tensor(outT[:, hB, qs], avsB,
                                                        rcB, Mult)
                            del acc[jc]

            # ---- phase 3: output projection (transposed) ----
            with tc.tile_pool(name="ostage", bufs=4) as ostage, \
                 tc.tile_pool(name="ps_o", bufs=4, space="PSUM") as ps_o:
                for sc in range(SC):
                    if sc == 1:
                        # deferred last-pair normalize: 8us DVE chain runs
                        # here while sc==1 drains go to ACT / outs to sync
                        hA, hB, qs, ssA, avsA, ssB, avsB = deferred
                        rcA = rpool.tile([128, 512], F32, tag="rc")
                        nc.vector.reciprocal(rcA, ssA)
                        nc.vector.tensor_tensor(outT_last[:, 0], avsA, rcA,
                                                Mult)
                        rcB = rpool.tile([128, 512], F32, tag="rc")
                        nc.vector.reciprocal(rcB, ssB)
                        nc.vector.tensor_tensor(outT_last[:, 1], avsB, rcB,
                                                Mult)
                    for et in range(ET):
                        po = ps_o.tile([128, 512], F32, tag="po")
                        for h in range(R):
                            if sc == SC - 1 and h >= 2:
                                mv = outT_last[:, h - 2]
                            else:
                                mv = outT[:, h, sc * 512 : (sc + 1) * 512]
                            nc.tensor.matmul(
                                po,
                                wo_sb[:, h, et * 128 : (et + 1) * 128],
                                mv,
                                start=(h == 0), stop=(h == R - 1),
                            )
                        st = ostage.tile([128, 512], BF16, tag="ost")
                        if et % 2 == 0 and sc >= 2:
                            nc.vector.tensor_copy(st, po)
                        else:
                            nc.scalar.activation(
                                st, po, mybir.ActivationFunctionType.Copy)
                        eng = (nc.sync if (sc * ET + et) % 2 == 0 or sc <= 1
                               else nc.scalar)
                        eng.dma_start(
                            otd[et * 128 : (et + 1) * 128,
                                sc * 512 : (sc + 1) * 512],
                            st,
                        )

    _split_multi_waits(nc)
    return nc


def _prepare(x, Wq, bq, Wk, bk, Wv, bv, Wo, bo):
    """Host-side sharding: build per-core input maps (bf16)."""
    import ml_dtypes

    bf16 = ml_dtypes.bfloat16
    x = np.asarray(x, dtype=np.float32)
    Wq = np.asarray(Wq, dtype=np.float32)
    bq = np.asarray(bq, dtype=np.float32)
    Wk = np.asarray(Wk, dtype=np.float32)
    bk = np.asarray(bk, dtype=np.float32)
    Wv = np.asarray(Wv, dtype=np.float32)
    bv = np.asarray(bv, dtype=np.float32)
    Wo = np.asarray(Wo, dtype=np.float32)

    isd = np.float32(1.0 / np.sqrt(D))

    xTs = [np.ascontiguousarray(x[b].T).astype(bf16) for b in range(B)]
    wqs = [
        np.ascontiguousarray(Wq[:, g * R * D : (g + 1) * R * D] * isd).astype(bf16)
        for g in range(G)
    ]
    def _pmajor(wmat):
        return np.ascontiguousarray(
            wmat.reshape(ET, 128, -1).transpose(1, 0, 2)).astype(bf16)

    wks = [_pmajor(Wk[:, g * D : (g + 1) * D]) for g in range(G)]
    wvs = [_pmajor(Wv[:, g * D : (g + 1) * D]) for g in range(G)]
    wos = [np.ascontiguousarray(Wo[g * R * D : (g + 1) * R * D, :]).astype(bf16)
           for g in range(G)]
    in_maps = []
    for core in range(8):
        b, g = divmod(core, G)
        in_maps.append({
            "xT": xTs[b],
            "wq": wqs[g],
            "wkh": wks[g],
            "wvh": wvs[g],
            "wo": wos[g],
            "bqv": bq[g * R * D : (g + 1) * R * D] * isd,
            "bkv": bk[g * D : (g + 1) * D],
            "bvv": bv[g * D : (g + 1) * D],
        })
    return in_maps


def _gather(results, bo):
    bo = np.asarray(bo, dtype=np.float32)
    out = np.empty((B, S, E), dtype=np.float32)
    for b in range(B):
        acc = results[b * G]["ot"].astype(np.float32)
        for g in range(1, G):
            acc += results[b * G + g]["ot"].astype(np.float32)
        out[b] = acc.T + bo
    return out


def kernel(x, Wq, bq, Wk, bk, Wv, bv, Wo, bo):
    from concourse.bass_utils import run_bass_kernel_spmd

    if "nc" not in _cache:
        _cache["nc"] = _build_program()
    nc = _cache["nc"]
    in_maps = _prepare(x, Wq, bq, Wk, bk, Wv, bv, Wo, bo)
    res = run_bass_kernel_spmd(nc, in_maps, core_ids=list(range(8)))
    return _gather(res.results, bo)
